# revision 19
# baseline (speedup 1.0000x reference)
"""Trainium2 Bass kernel for nn_AttentionDigitCaps (capsule dynamic routing).

reference math:
    x = inputs.reshape(B, N, iL)                      # B=32, N=2048, iL=32
    u = einsum('bji,jik->bjk', x, W).reshape(B,N,C,L) # C=L=32
    b = 0; for r in 3: c = softmax(b, C); s = sum_j u*c + biases; v = squash(s)
                       if r<2: b += sum_l u*v

Two launches (instead of one per routing iteration):

Launch A (capsule-sharded, 256 j per core): u = x @ W in bf16 streamed out
  to DRAM (16.8 MB/core) plus the s0 partial (sum_j u).  bf16 W halves the
  HBM traffic vs f32 and runs the PE at the full bf16 rate.
Host: reduce s0 across cores, v1 = squash(s0/C + bias) in f64, and
  all-to-all the u tensor from capsule-sharded to batch-sharded layout.
Launch B (batch-sharded, 4 b per core): all remaining routing math is
  batch-local, so BOTH remaining iterations run in one launch from
  SBUF-resident u (read once, 16.8 MB/core): b=sum_l u*v; c=softmax(b);
  s=sum_j c*u; v=squash(s) on-core; output v3 directly.

Launch B layout: partition p=(j32,b4) [j32 = an arbitrary 32-way capsule
  split, b4 = local batch], free dims (jj in 64, (l,c) with c inner).
  sum_l -> in-place bf16 add-tree (c contiguous); softmax over c -> ACT exp
  + DVE reduce (c innermost); sum_j -> PE matmul with a tiled eye(4)
  selector (contracts j32 over partitions, psum-accumulates over jj); v
  broadcast to 128 partitions via a tiny K=4 replicator matmul.  The final
  iteration emits the raw s2 partial; the host does the last squash in f64.

All big DVE ops are bf16 with contiguous innermost runs: TRN2's vector
engine caps tensor-tensor ops at ~1.9 elem/ns/partition (2x_1port), and
the two muls + l-tree per iteration are the throughput floor of launch B.
"""

import os
import sys
import numpy as np

if "/opt/trn_rl_repo" not in sys.path:
    sys.path.insert(0, "/opt/trn_rl_repo")

import ml_dtypes

BF16 = ml_dtypes.bfloat16

CORES = 8
B, N, IL, C, L = 32, 2048, 32, 32, 32
NLOC = N // CORES          # 256 capsules per core (launch A)
BLOC = B // CORES          # 4 batch items per core (launch B)
CL = C * L                 # 1024
JJ = N // 32               # 64 j-chunks in launch B
EPS = 1e-7

_CACHE = {}


def _mk_nc():
    from concourse import bacc
    return bacc.Bacc("TRN2", target_bir_lowering=False, debug=False,
                     num_devices=CORES)


def _build_A():
    """u[b, j_local, cl] (bf16, DRAM) + s0_partial = sum_j u  -> [B, CL] f32.

    j_local = c4*64 + g2*16 + a*4 + jc; partition for matmul K = (a, i),
    output partition M = (a, b).  xbd is the host-built block-diagonal x
    (zero where the a of K differs from the a of M) so one 128x128 matmul
    computes 4 capsules' per-capsule predictions at once.
    """
    from concourse import tile
    import concourse.mybir as mybir

    f32 = mybir.dt.float32
    bf16 = mybir.dt.bfloat16
    AF = mybir.ActivationFunctionType

    nc = _mk_nc()
    xbd_p = nc.dram_tensor("xbd", [128, 16, 4, 128], bf16, kind="ExternalInput")
    w_p = nc.dram_tensor("w", [4, 128, 16, CL], bf16, kind="ExternalInput")
    bones_p = nc.dram_tensor("bones", [128, B], bf16, kind="ExternalInput")
    u_out = nc.dram_tensor("u", [4, 128, 16, CL], bf16, kind="ExternalOutput")
    s0_out = nc.dram_tensor("s0", [B, CL], f32, kind="ExternalOutput")

    with tile.TileContext(nc) as tc:
        with (
            tc.tile_pool(name="const", bufs=1) as constp,
            tc.tile_pool(name="wstream", bufs=3) as wp,
            tc.tile_pool(name="ustream", bufs=2) as up,
            tc.tile_pool(name="eps", bufs=2, space="PSUM") as epsp,
            tc.tile_pool(name="acc", bufs=1, space="PSUM") as accp,
        ):
            xbd = constp.tile([128, 16, 4, 128], bf16)
            bones = constp.tile([128, B], bf16)

            s0_ps = accp.tile([B, CL], f32, tag="s0acc")
            # 8 half-chunks of 8 capsule-columns; W prefetched 2 chunks ahead
            # of the evac-dependent u_out stores so the in-order DMA queue
            # never stalls the next W load
            w_tiles = [wp.tile([128, 8, CL], bf16, tag="w", name=f"w_t{hc}")
                       for hc in range(8)]

            def wdma(hc):
                nc.sync.dma_start(out=w_tiles[hc][:],
                                  in_=w_p[hc // 2, :, 8 * (hc % 2):
                                          8 * (hc % 2) + 8, :])

            # first compute needs only w0 + the first xbd half: issue those
            # ahead of the rest so the pipeline starts ~6us earlier
            wdma(0)
            nc.sync.dma_start(out=xbd[:, 0:4], in_=xbd_p[:, 0:4])
            wdma(1)
            nc.sync.dma_start(out=xbd[:, 4:16], in_=xbd_p[:, 4:16])
            nc.sync.dma_start(out=bones[:], in_=bones_p[:])
            for hc in range(8):
                w_t = w_tiles[hc]
                if hc + 2 < 8:
                    wdma(hc + 2)
                u_sb = up.tile([128, 8, CL], bf16, tag="u")
                for g2 in range(2):
                    for jc in range(4):
                        g = hc * 2 + g2
                        m = g2 * 4 + jc
                        ps = epsp.tile([128, CL], f32, tag="ups")
                        for h in range(2):
                            nc.tensor.matmul(
                                ps[:, 512 * h:512 * h + 512],
                                xbd[:, g, jc, :],
                                w_t[:, m, 512 * h:512 * h + 512],
                                start=True, stop=True)
                        # evacuate psum -> bf16 SBUF, alternating engines
                        if m % 2 == 0:
                            nc.scalar.activation(u_sb[:, m, :], ps[:], AF.Copy)
                        else:
                            nc.vector.tensor_copy(u_sb[:, m, :], ps[:])
                # s0 partial: bones.T @ u (contracts a, keeps b) for the chunk
                for m in range(8):
                    for h in range(2):
                        nc.tensor.matmul(
                            s0_ps[:, 512 * h:512 * h + 512],
                            bones[:], u_sb[:, m, 512 * h:512 * h + 512],
                            start=(hc == 0 and m == 0),
                            stop=(hc == 7 and m == 7),
                            skip_group_check=True)
                mo = 8 * (hc % 2)
                nc.sync.dma_start(out=u_out[hc // 2, :, mo:mo + 4, :],
                                  in_=u_sb[:, 0:4, :])
                nc.sync.dma_start(out=u_out[hc // 2, :, mo + 4:mo + 8, :],
                                  in_=u_sb[:, 4:8, :])

            s0_loc = constp.tile([B, CL], f32)
            nc.scalar.activation(s0_loc[:], s0_ps[:], AF.Copy)
            nc.sync.dma_start(out=s0_out[:], in_=s0_loc[:])

    nc.compile()
    return nc


def _build_B():
    """Routing iterations 1 and 2 for 4 local batch items, all capsules."""
    from concourse import tile
    import concourse.mybir as mybir

    f32 = mybir.dt.float32
    bf16 = mybir.dt.bfloat16
    AF = mybir.ActivationFunctionType
    OP = mybir.AluOpType
    AX = mybir.AxisListType

    nc = _mk_nc()
    u_p = nc.dram_tensor("u", [128, JJ, CL], bf16, kind="ExternalInput")
    vrep_p = nc.dram_tensor("vrep", [128, CL], bf16, kind="ExternalInput")
    selw_p = nc.dram_tensor("selw", [128, 4], bf16, kind="ExternalInput")
    repw_p = nc.dram_tensor("repw", [4, 128], bf16, kind="ExternalInput")
    bias4_p = nc.dram_tensor("bias4", [4, CL], f32, kind="ExternalInput")
    vout_p = nc.dram_tensor("vout", [4, CL], f32, kind="ExternalOutput")

    NK = 8          # jj-chunks
    KJ = JJ // NK   # jj per chunk

    with tile.TileContext(nc) as tc:
        with (
            tc.tile_pool(name="const", bufs=1) as constp,
            tc.tile_pool(name="ub", bufs=1) as ubp,
            tc.tile_pool(name="work", bufs=1) as workp,
            tc.tile_pool(name="small", bufs=1) as smallp,
            tc.tile_pool(name="sps", bufs=2, space="PSUM") as psp,
            tc.tile_pool(name="vps", bufs=1, space="PSUM") as vpsp,
        ):
            vrep1 = constp.tile([128, CL], bf16)
            selw = constp.tile([128, 4], bf16)
            repw = constp.tile([4, 128], bf16)
            bias4 = constp.tile([4, CL], f32)
            # chunk schedule: small chunks at the start (compute begins after
            # a 1 MB DMA instead of 2 MB) and at the end (shorter uncovered
            # softmax tail after the last chunk)
            CHUNKS = ([(0, 4), (4, 4)] +
                      [(8 + 8 * i, 8) for i in range(6)] +
                      [(56, 4), (60, 4)])
            u_tiles = []
            for ci, (j0, szc) in enumerate(CHUNKS):
                ut = ubp.tile([128, szc, CL], bf16, tag=f"u{ci}",
                              name=f"ut{ci}")
                nc.sync.dma_start(out=ut[:], in_=u_p[:, j0:j0 + szc, :])
                if ci == 0:
                    nc.sync.dma_start(out=vrep1[:], in_=vrep_p[:])
                    nc.sync.dma_start(out=selw[:], in_=selw_p[:])
                    nc.sync.dma_start(out=repw[:], in_=repw_p[:])
                    nc.sync.dma_start(out=bias4[:], in_=bias4_p[:])
                u_tiles.append(ut)

            b_state = constp.tile([128, JJ, C], f32)
            warm = constp.tile([4, 4], f32)
            nc.scalar.activation(warm[:], bias4[:, 0:4], AF.Exp)
            nc.scalar.activation(warm[:], bias4[:, 0:4], AF.Sqrt)
            vcur = vrep1

            for it in range(2):
                s_ps = psp.tile([4, CL], f32, tag="sps")

                def flush(ci, szc, e):
                    """softmax tail + s accumulation for chunk ci."""
                    z = workp.tile([128, KJ], f32, tag="z")
                    nc.vector.tensor_reduce(z[:, 0:szc], e, axis=AX.X,
                                            op=OP.add)
                    rz = workp.tile([128, KJ], f32, tag="rz")
                    nc.vector.reciprocal(rz[:, 0:szc], z[:, 0:szc])
                    cw = workp.tile([128, KJ, C], bf16, tag="cw")
                    rzb = rz[:, 0:szc].rearrange("p (j x) -> p j x", x=1)
                    rzb = rzb.broadcast_to([128, szc, C])
                    nc.vector.tensor_mul(cw[:, 0:szc], e, rzb)
                    tmpf = workp.tile([128, KJ, L, C], bf16, tag="tmp")
                    # on the last chunk, emit tmp in two halves so the final
                    # PE matmuls drain while the second half still multiplies
                    halves = ([(0, szc)] if ci != len(CHUNKS) - 1 else
                              [(0, szc // 2), (szc // 2, szc - szc // 2)])
                    for jh, hs in halves:
                        tmp = tmpf[:, jh:jh + hs]
                        uc = u_tiles[ci][:, jh:jh + hs, :].rearrange(
                            "p j (l c) -> p j l c", l=L)
                        cwb = cw[:, jh:jh + hs].rearrange(
                            "p j (x c) -> p j x c", x=1)
                        cwb = cwb.broadcast_to([128, hs, L, C])
                        nc.vector.tensor_mul(tmp, uc, cwb)
                        for jl in range(hs):
                            jj = jh + jl
                            rhs = tmp[:, jl].rearrange("p l c -> p (l c)")
                            for h in range(2):
                                nc.tensor.matmul(
                                    s_ps[:, 512 * h:512 * h + 512],
                                    selw[:], rhs[:, 512 * h:512 * h + 512],
                                    start=(ci == 0 and jj == 0),
                                    stop=(ci == len(CHUNKS) - 1 and
                                          jj == szc - 1),
                                    skip_group_check=True)

                carry = None
                for ci, (j0, szc) in enumerate(CHUNKS):
                    uc = u_tiles[ci].rearrange("p j (l c) -> p j l c", l=L)
                    t0f = workp.tile([128, KJ, L, C], bf16, tag="t0")
                    t0 = t0f[:, 0:szc]
                    vb = vcur.rearrange("p (x l c) -> p x l c", x=1, l=L)
                    vb = vb.broadcast_to([128, szc, L, C])
                    nc.vector.tensor_mul(t0, uc, vb)
                    # sum over l: in-place bf16 add-tree, c contiguous
                    for hw in (16, 8, 4, 2):
                        nc.vector.tensor_add(t0[:, :, 0:hw, :],
                                             t0[:, :, 0:hw, :],
                                             t0[:, :, hw:2 * hw, :])
                    bc = b_state[:, j0:j0 + szc, :]
                    if it == 0:
                        nc.vector.tensor_add(bc, t0[:, :, 0, :], t0[:, :, 1, :])
                    else:
                        r5 = workp.tile([128, KJ, C], bf16, tag="r5")
                        nc.vector.tensor_add(r5[:, 0:szc], t0[:, :, 0, :],
                                             t0[:, :, 1, :])
                        nc.vector.tensor_add(bc, bc, r5[:, 0:szc])
                    e_f = workp.tile([128, KJ, C], bf16, tag="e", bufs=2)
                    nc.scalar.activation(e_f[:, 0:szc], bc, AF.Exp)
                    if carry is not None:
                        flush(*carry)
                    carry = (ci, szc, e_f[:, 0:szc])
                    if it == 0 and ci == len(CHUNKS) - 1:
                        nc.scalar.activation(warm[:], bias4[0:4, 0:4],
                                             AF.Sqrt)
                flush(*carry)

                if it == 1:
                    # raw s2 partial -> host does bias + squash in f64
                    s_raw = smallp.tile([4, CL], f32, tag="sraw")
                    nc.vector.tensor_copy(s_raw[:], s_ps[:])
                    nc.sync.dma_start(out=vout_p[:], in_=s_raw[:])
                    continue
                # s = s_psum + bias; v = squash(s) on partitions 0..3
                s_sb = smallp.tile([4, CL], f32, tag="s")
                nc.vector.tensor_add(s_sb[:], s_ps[:], bias4[:])
                q2 = smallp.tile([4, CL], f32, tag="q2")
                nc.vector.tensor_mul(q2[:], s_sb[:], s_sb[:])
                q2v = q2.rearrange("p (l c) -> p l c", l=L)
                for hw in (16, 8, 4, 2):
                    nc.vector.tensor_add(q2v[:, 0:hw, :], q2v[:, 0:hw, :],
                                         q2v[:, hw:2 * hw, :])
                qs = smallp.tile([4, C], f32, tag="qs")
                nc.vector.tensor_add(qs[:], q2v[:, 0, :], q2v[:, 1, :])
                nrm = smallp.tile([4, C], f32, tag="nrm")
                nc.scalar.activation(nrm[:], qs[:], AF.Sqrt)
                q1 = smallp.tile([4, C], f32, tag="q1")
                nc.vector.tensor_scalar_add(q1[:], qs[:], 1.0)
                den = smallp.tile([4, C], f32, tag="den")
                nc.vector.scalar_tensor_tensor(
                    out=den[:], in0=nrm[:], scalar=EPS, in1=q1[:],
                    op0=OP.add, op1=OP.mult)
                rden = smallp.tile([4, C], f32, tag="rden")
                nc.vector.reciprocal(rden[:], den[:])
                fac = smallp.tile([4, C], f32, tag="fac")
                nc.vector.tensor_mul(fac[:], qs[:], rden[:])
                vb16 = smallp.tile([4, CL], bf16, tag="vb16")
                facb = fac.rearrange("p (x c) -> p x c", x=1)
                facb = facb.broadcast_to([4, L, C])
                nc.vector.tensor_mul(
                    vb16.rearrange("p (l c) -> p l c", l=L),
                    s_sb.rearrange("p (l c) -> p l c", l=L), facb)
                vps = vpsp.tile([128, CL], f32, tag="vrep")
                for h in range(2):
                    nc.tensor.matmul(
                        vps[:, 512 * h:512 * h + 512],
                        repw[:], vb16[:, 512 * h:512 * h + 512],
                        start=True, stop=True)
                vrep2 = constp.tile([128, CL], bf16)
                nc.scalar.activation(vrep2[:], vps[:], AF.Copy)
                vcur = vrep2

    nc.compile()
    return nc


def _host_prep_A(inputs, W):
    """Build per-core bf16 inputs for launch A."""
    x = np.ascontiguousarray(inputs.reshape(B, N, IL), dtype=np.float32)
    # x_sh[r, (a,i), g, jc, b] = x[b, r*256+g*16+a*4+jc, i]
    xr = x.reshape(B, CORES, 16, 4, 4, IL)
    x_sh = xr.transpose(1, 3, 5, 2, 4, 0).reshape(CORES, 128, 16, 4, B)
    xbd = np.zeros((CORES, 128, 16, 4, 128), np.float32)
    for a in range(4):
        xbd[:, 32 * a:32 * a + 32, :, :, 32 * a:32 * a + 32] = \
            x_sh[:, 32 * a:32 * a + 32]
    xbd = np.ascontiguousarray(xbd).astype(BF16)
    # w_sh[r, c4, (a,i), (g2,jc), cl] = W[r*256+(c4*4+g2)*16+a*4+jc, i, cl]
    wr = np.asarray(W, np.float32).reshape(CORES, 4, 4, 4, 4, IL, C, L)
    w_sh = np.ascontiguousarray(
        wr.transpose(0, 1, 3, 5, 2, 4, 7, 6).reshape(CORES, 4, 128, 16, CL)
    ).astype(BF16)
    bones = np.ascontiguousarray(
        np.tile(np.eye(B, dtype=np.float32), (4, 1))).astype(BF16)
    return xbd, w_sh, bones


def _squash_np(s):
    """reference squash in float64; s is [B, C, L]."""
    s = s.astype(np.float64)
    n = np.linalg.norm(s, axis=-1, keepdims=True)
    return (n ** 2 / (1 + n ** 2) / (n + EPS)) * s


def _install_trace_hook():
    """Register the NTFF profiling hook (antenv.axon_hooks is absent in this
    container, but the ctypes implementation ships in trn_agent_boot)."""
    import types

    if "antenv.axon_hooks" in sys.modules:
        return
    try:
        from trn_agent_boot.trn_boot import _ntff_profile_via_ctypes
        hook = _ntff_profile_via_ctypes("/opt/axon/libaxon_pjrt.so")
        if hook is None:
            return
        m = types.ModuleType("antenv.axon_hooks")
        m.get_axon_ntff_profile_hook = lambda: hook
        sys.modules["antenv.axon_hooks"] = m
        from concourse import bass_utils
        bass_utils.upload_artifacts = lambda tmpdir: tmpdir  # no egress
    except Exception as e:  # profiling is best-effort
        print(f"trace hook install failed: {e}", file=sys.stderr)


def kernel(inputs, W, biases):
    from concourse.bass_utils import run_bass_kernel_spmd

    if "ga" not in _CACHE:
        _CACHE["ga"] = _build_A()
        _CACHE["gb"] = _build_B()
    ga, gb = _CACHE["ga"], _CACHE["gb"]

    xbd, w_sh, bones = _host_prep_A(inputs, W)
    biases64 = np.asarray(biases, dtype=np.float64)
    trace = os.environ.get("KERNEL_TRACE", "0") == "1"
    if trace:
        _install_trace_hook()
    cores = list(range(CORES))
    results = []

    def launch(nc, maps):
        res = run_bass_kernel_spmd(nc, maps, core_ids=cores, trace=trace)
        results.append(res)
        return res.results

    # --- launch A: u (bf16) + s0 partials --------------------------------
    rA = launch(ga, [{"xbd": xbd[r], "w": w_sh[r], "bones": bones}
                     for r in cores])
    s0 = sum(np.asarray(rA[r]["s0"], np.float64) for r in cores)
    v1 = _squash_np(s0.reshape(B, L, C).transpose(0, 2, 1) / C + biases64)

    # --- host: capsule-shard -> batch-shard all-to-all of u --------------
    # u_a[q] is [c4, (a,b), (g2,jc), cl]; local j = c4*64 + g2*16 + a*4 + jc
    blocks = []
    for q in cores:
        ua = np.asarray(rA[q]["u"]).view(np.uint16)
        ua = ua.reshape(4, 4, 32, 4, 4, CL).transpose(0, 3, 1, 4, 2, 5)
        blocks.append(ua.reshape(NLOC, B, CL))
    U = np.concatenate(blocks, axis=0)          # [N, B, CL] (uint16 view)
    U = U.reshape(JJ, 32, B, CL)                # [jj, j32, b, cl]

    v1f = np.ascontiguousarray(
        v1.transpose(0, 2, 1).reshape(B, CL).astype(np.float32))
    selw = np.ascontiguousarray(
        np.tile(np.eye(4, dtype=np.float32), (32, 1))).astype(BF16)
    repw = np.ascontiguousarray(
        np.tile(np.eye(4, dtype=np.float32), (1, 32))).astype(BF16)
    bias4 = np.ascontiguousarray(
        np.tile(np.asarray(biases, np.float32).T.reshape(1, CL), (4, 1)))

    mapsB = []
    for r in cores:
        ub = np.ascontiguousarray(
            U[:, :, 4 * r:4 * r + 4, :].transpose(1, 2, 0, 3)
            .reshape(128, JJ, CL)).view(BF16)
        vrep = np.ascontiguousarray(
            np.tile(v1f[4 * r:4 * r + 4], (32, 1))).astype(BF16)
        mapsB.append({"u": ub, "vrep": vrep, "selw": selw, "repw": repw,
                      "bias4": bias4})

    # --- launch B: routing iterations 1+2, batch-local -------------------
    rB = launch(gb, mapsB)
    v = np.empty((B, C, L), np.float32)
    for r in cores:
        s2 = np.asarray(rB[r]["vout"], np.float64).reshape(
            BLOC, L, C).transpose(0, 2, 1)
        v[4 * r:4 * r + 4] = _squash_np(s2 + biases).astype(np.float32)

    _CACHE["last_results"] = results
    return np.ascontiguousarray(v)
